# revision 66
# baseline (speedup 1.0000x reference)
"""Trainium2 Bass kernel for GQA attention block (nn_Attention_36627481101235).

Reference computation (BS=1, SEQ=2048, DIM=4096, 32 q-heads, 8 kv-heads,
head_dim=128):
    q/k/v projections -> interleaved RoPE on q,k -> repeat_kv -> causal
    softmax attention -> output projection.

Sharding: tensor-parallel by heads over 8 cores. Core c gets q-heads
4c..4c+3 and kv-head c (GQA groups stay intact). Each core computes its
partial out = attn_out_c @ wo_c; the host sums the 8 bf16 partials in
f32. All matmul inputs are bf16 (x, weights, rotated q/k, P, v) with f32
psum accumulation; RoPE tables stay f32 (the host pre-permutes wq/wk
columns so RoPE pairs are contiguous 64-row halves).

Structure (per core). Engine streams execute IN ORDER, so overlap quality
is set by static emission interleaving:

  Projections. Weights live in one combined tensor w6 = [k|v|q0..q3] per
  k-tile (1.5KB descriptors). Chunk 0 runs k-step-outer (all 6 outputs per
  k-tile) so compute tracks the w6/x DMA arrival, with the last 8 k-tiles
  output-major so evacuations start early. Chunks 1-3 run OUTPUT-outer
  (k, v, q0..q3, each a 32-matmul psum accumulation): only 3 psum banks
  held (pacc ring), freeing banks for the attention phase to overlap.

  Attention B(sc) is emitted as a stream of micro-ops (per-kt scores+exp,
  per-kt PV+denominator, finalize) WOVEN between the NEXT chunk's
  projection matmuls at a fixed ratio, so the ACT exp latency (the real
  pacer: ~670ns per [128,512] tile vs 213ns of PE per matmul) hides under
  projection work. B(3) weaves with phase C the same way.

  Softmax denominator: instead of ones^T @ P row-sums (512 moving columns
  = as expensive as PV), each P subtile is the STATIONARY operand with a
  [128,1] ones moving operand: den columns [q,1] accumulate in a [128,4]
  psum at ~1 cycle per matmul (all 4 columns form ONE psum accumulation
  group — one pending group per bank). finalize (all fp16 to dodge the
  f32r narrow-matmul 4x penalty): reciprocal (DVE, psum-direct) -> PE
  transpose [128,4]->[4,128] -> one ACT copy to sbuf (partition base 0;
  per-row reads at partitions 1..3 are illegal) -> four K=4 selector
  matmuls broadcast 1/den to [128,512] -> DVE-multiply into aoT.

  Causal handling: key tiles above the diagonal are skipped; on diagonal
  tiles the dead columns [0, 128a) are sliced out; a single 128x128
  additive tril mask covers the boundary block. (Cost is per moving
  COLUMN, so 128-granularity slicing is already column-optimal.)

  Phase C: out[s,:] = attn_outT.T @ wo, psum->sbuf copies on DVE (ACT is
  busy with B(3) exps), one 2MB DMA per 128-row s-tile; the last tile
  drains in two bulks and the final dc runs as two [128,256] psum groups
  so the post-last-matmul chain is short. wo is DMA'd into the x-chunk
  ring slot freed by chunk 2.

  DMA: w6 on the SP queue, chunk-0 x on the ACT HWDGE queue (their
  dma_start sequencing costs ~600ns each and would serialize on one
  queue); granule ramp 1,1,2 then 2-kt granules (finer granules track the
  chunk-0 wave's 1278ns/kt consumption); rope tables/consts issued after
  all granules (they'd otherwise delay the wave's last x tiles).

PSUM banks (8): pacc x3 (projection accumulators, phase-C po), pS x3
(scores ring, shared with the 1/den broadcast and the v-transpose
staging), pao x1 (PV accumulator), pdq x1 (denominator + recip
transpose).

TimelineSim: 347626 ns per core (PE busy 336.1us = algorithmic floor at
1 bf16 column/cycle, 96.7% busy; baseline was 427681 ns). Remaining idle
is protocol-bound: ~3us startup DMA latency, ~4us final DMA + drain
barrier, ~2.7us instruction-boundary slivers. Measured end-to-end
relative error vs the fp32 reference ~7.1e-3.
"""
import numpy as np

import concourse.mybir as mybir
import concourse.tile as tile
from concourse import bacc

BS, SEQ, DIM = 1, 2048, 4096
NH, DH = 4, 128          # q-heads per core, head dim
DQ = NH * DH             # 512
NCORES = 8
P = 128                  # partitions
SC = 512                 # s-chunk width
NSC = SEQ // SC          # 4
NKT = DIM // P           # 32 contraction tiles for projections
W6 = 2 * DH + DQ         # 768 combined weight cols per k-tile: k|v|q0..q3
F32R = mybir.dt.float32r
F16 = mybir.dt.float16
F32 = mybir.dt.float32
BF16 = mybir.dt.bfloat16
NEG = -1e9
# output order within a projection chunk; j indexes this list
OUTS = ("k", "v", "q0", "q1", "q2", "q3")
WOFF = {"k": 0, "v": DH, "q0": 2 * DH, "q1": 3 * DH, "q2": 4 * DH,
        "q3": 5 * DH}


def build_nc(num_devices=NCORES):
    nc = bacc.Bacc("TRN2", target_bir_lowering=False, debug=False,
                   enable_asserts=False, num_devices=num_devices)
    xT = nc.dram_tensor("xT", (DIM, SEQ), BF16, kind="ExternalInput").ap()
    w6 = nc.dram_tensor("w6", (DIM, W6), BF16, kind="ExternalInput").ap()
    wo = nc.dram_tensor("wo", (DQ, DIM), BF16, kind="ExternalInput").ap()
    ropeA = nc.dram_tensor("ropeA", (P, SEQ), F32R, kind="ExternalInput").ap()
    ropeB = nc.dram_tensor("ropeB", (P, SEQ), F32R, kind="ExternalInput").ap()
    masks = nc.dram_tensor("masks", (P, P), BF16, kind="ExternalInput").ap()
    ones_col = nc.dram_tensor("ones_col", (1, P), F32R, kind="ExternalInput").ap()
    ones128 = nc.dram_tensor("ones128", (P, 1), BF16, kind="ExternalInput").ap()
    ident = nc.dram_tensor("ident", (P, P), BF16, kind="ExternalInput").ap()
    identf = nc.dram_tensor("identf", (P, P), F16, kind="ExternalInput").ap()
    sel4 = nc.dram_tensor("sel4", (4, DQ), F16, kind="ExternalInput").ap()
    out = nc.dram_tensor("out", (SEQ, DIM), BF16, kind="ExternalOutput").ap()

    with tile.TileContext(nc) as tc:
        from contextlib import ExitStack
        with tc.tile_pool(name="persist", bufs=1) as pp, \
             tc.tile_pool(name="pacc", bufs=3, space="PSUM") as pacc, \
             tc.tile_pool(name="pS", bufs=3, space="PSUM") as pS, \
             tc.tile_pool(name="pao", bufs=1, space="PSUM") as pao, \
             tc.tile_pool(name="pdq", bufs=1, space="PSUM") as pdq, \
             tc.tile_pool(name="qTc_p", bufs=2) as qTc_p, \
             tc.tile_pool(name="tmp_p", bufs=2) as tmp_p, \
             tc.tile_pool(name="pP_p", bufs=16) as pP_p, \
             tc.tile_pool(name="rec_p", bufs=2) as rec_p, \
             tc.tile_pool(name="rbs_p", bufs=2) as rbs_p, \
             tc.tile_pool(name="vt_p", bufs=2) as vt_p:
            kT_sb = pp.tile([P, SEQ], BF16)             # rotated K^T [d, s]
            v_sb = pp.tile([P, SEQ], BF16)              # v tiles [s%128, st*128+d]
            aoT_sb = pp.tile([P, NH * SEQ], BF16)       # attn_outT [d, h*SEQ+s]
            ones_col_sb = pp.tile([1, P], F32R)
            ones128_sb = pp.tile([P, 1], BF16)
            ident_sb = pp.tile([P, P], BF16)
            identf_sb = pp.tile([P, P], F16)
            sel4_sb = pp.tile([4, DQ], F16)

            inner = ExitStack()
            w6_p = inner.enter_context(tc.tile_pool(name="w6_p", bufs=1))
            xt_p = inner.enter_context(tc.tile_pool(name="xt_p", bufs=2))
            tab_p = inner.enter_context(tc.tile_pool(name="tab_p", bufs=1))
            w6_sb = w6_p.tile([P, NKT * W6], BF16)
            ropeA_sb = tab_p.tile([P, SEQ], F32R, tag="ra")
            ropeB_sb = tab_p.tile([P, SEQ], F32R, tag="rb")
            masks_sb = tab_p.tile([P, P], BF16, tag="mk")

            # 3D views for batched k-tile DMAs: [p, ktile, width]
            xT3 = xT.rearrange("(t p) m -> p t m", p=P)
            w63 = w6.rearrange("(t p) m -> p t m", p=P)
            w6_sb3 = w6_sb[:].rearrange("p (t m) -> p t m", m=W6)

            def load_x(sc, granules=None):
                """DMA one chunk of x^T as [P, NKT*SC] in 4-kt granules."""
                xt = xt_p.tile([P, NKT * SC], BF16, tag="xt",
                               name=f"xt{sc}")
                xt3 = xt[:].rearrange("p (t m) -> p t m", m=SC)
                for g in range(NKT // 4):
                    ksl = slice(4 * g, 4 * g + 4)
                    nc.sync.dma_start(xt3[:, ksl, :],
                                      xT3[:, ksl, sc * SC:(sc + 1) * SC])
                    if granules is not None:
                        granules(g)
                return xt

            def rope_evac(ps_tile, dst_ap, sc, uid):
                """dst = RoPE(ps_tile) on DVE (cross-partition reads are
                legal when one operand is PSUM)."""
                cols = slice(sc * SC, (sc + 1) * SC)
                swp = tmp_p.tile([P, SC], F32, tag="ropeswp",
                                 name=f"swp{uid}")
                nc.vector.tensor_mul(swp[0:64, :], ps_tile[64:128, :],
                                     ropeB_sb[0:64, cols])
                nc.vector.tensor_mul(swp[64:128, :], ps_tile[0:64, :],
                                     ropeB_sb[64:128, cols])
                nc.vector.tensor_mul(ps_tile[:], ps_tile[:],
                                     ropeA_sb[:, cols])
                nc.vector.tensor_add(dst_ap, ps_tile[:], swp[:])

            def v_evac(ps_tile, sc):
                """v_sb[:, sc*4P:(sc+1)*4P] = transpose(v psum) via PE."""
                vtmp = vt_p.tile([P, SC], BF16, tag="vtmp",
                                 name=f"vtmp{sc}")
                nc.scalar.copy(vtmp[:], ps_tile[:])
                ptr = pS.tile([P, 4 * P], BF16, tag="S",
                              name=f"ptr{sc}")
                for t in range(4):
                    nc.tensor.transpose(ptr[:, t * P:(t + 1) * P],
                                        vtmp[:, t * P:(t + 1) * P],
                                        ident_sb[:])
                nc.scalar.copy(v_sb[:, sc * SC:(sc + 1) * SC], ptr[:])

            def evac(j, sc, ps_tile, qTc):
                if j == "k":
                    rope_evac(ps_tile, kT_sb[:, sc * SC:(sc + 1) * SC],
                              sc, f"{sc}_k")
                elif j == "v":
                    v_evac(ps_tile, sc)
                else:
                    h = int(j[1])
                    rope_evac(ps_tile, qTc[:, h * SC:(h + 1) * SC],
                              sc, f"{sc}_{h}")

            # ---------------- attention micro-op streams ----------------
            def finalize(h, sc, ao, dcolT):
                """normalize head h's attn_outT by 1/denominator. dcolT is
                [128, 4] (den for q=128j+p in column j). Two-part emission
                (a: recip+transpose, b: gather+broadcast+mul) so PE parts
                sit a bit apart in the stream."""
                def part_a():
                    rec4 = rec_p.tile([P, 4], F16, tag="rec",
                                      name=f"rec{sc}_{h}")
                    with nc.allow_low_precision(reason="softmax denom"):
                        nc.vector.reciprocal(rec4[:], dcolT[:])
                    tp = pdq.tile([P, P], F16, tag="dq",
                                  name=f"tp{sc}_{h}")
                    nc.tensor.transpose(tp[0:4, 0:P], rec4[:],
                                        identf_sb[:])
                    return tp

                def part_b(tp):
                    tps = rec_p.tile([4, P], F16, tag="recrow",
                                     name=f"recrow{sc}_{h}")
                    nc.scalar.copy(tps[:], tp[0:4, 0:P])
                    rb = pS.tile([P, SC], F32, tag="S",
                                 name=f"rb{sc}_{h}")
                    for jj in range(4):
                        nc.tensor.matmul(rb[:, jj * P:(jj + 1) * P],
                                         sel4_sb[:, jj * P:(jj + 1) * P],
                                         tps[:], start=True, stop=True)
                    rb_sb = rbs_p.tile([P, SC], F32, tag="rbsb",
                                       name=f"rbsb{sc}_{h}")
                    nc.scalar.copy(rb_sb[:], rb[:])
                    nc.vector.tensor_mul(
                        aoT_sb[:, h * SEQ + sc * SC:h * SEQ + (sc + 1) * SC],
                        ao[:], rb_sb[:])
                return part_a, part_b

            def B_stream(sc, qTc):
                """Yield attention micro-op callables for chunk sc."""
                nkt = 4 * sc + 4

                def lo_of(kt):
                    return 128 * (kt - 4 * sc) if kt >= 4 * sc else 0

                for h in range(NH):
                    Pts = []
                    st = {}

                    def S_op(h, kt, Pts=None):
                        lo = lo_of(kt)
                        S = pS.tile([P, SC], F32, tag="S",
                                    name=f"S{sc}_{h}_{kt}")
                        nc.tensor.matmul(
                            S[:, lo:], kT_sb[:, kt * P:(kt + 1) * P],
                            qTc[:, h * SC + lo:(h + 1) * SC],
                            start=True, stop=True)
                        if kt >= 4 * sc:
                            nc.vector.tensor_add(
                                S[:, lo:lo + P], S[:, lo:lo + P], masks_sb[:])
                        Pt = pP_p.tile([P, SC], BF16, tag="P",
                                       name=f"P{sc}_{h}_{kt}")
                        nc.scalar.activation(
                            Pt[:, lo:], S[:, lo:],
                            mybir.ActivationFunctionType.Exp)
                        Pts.append(Pt)

                    def C_op(h, kt, st=None, Pts=None):
                        lo = lo_of(kt)
                        if kt == 0:
                            st["ao"] = pao.tile([P, SC], F32, tag="ao",
                                                name=f"ao{sc}_{h}")
                            st["dcolT"] = pdq.tile([P, 4], F32, tag="dq",
                                                   name=f"dcol{sc}_{h}")
                        nc.tensor.matmul(
                            st["ao"][:, lo:], v_sb[:, kt * P:(kt + 1) * P],
                            Pts[kt][:, lo:],
                            start=(kt == 0), stop=(kt == nkt - 1))
                        # denominator: P subtile stationary, ones moving.
                        # All 4 columns form ONE psum accumulation group
                        # (one pending group per bank).
                        for jj in range(max(0, kt - 4 * sc), 4):
                            nc.tensor.matmul(
                                st["dcolT"][:, jj:jj + 1],
                                Pts[kt][:, jj * P:(jj + 1) * P],
                                ones128_sb[:],
                                start=(kt == 0 and jj == 0),
                                stop=(kt == nkt - 1 and jj == 3))

                    for kt in range(nkt):
                        yield (lambda h=h, kt=kt, Pts=Pts: S_op(h, kt, Pts))
                    for kt in range(nkt):
                        yield (lambda h=h, kt=kt, st=st, Pts=Pts:
                               C_op(h, kt, st, Pts))

                    def fin_a(h=h, st=st):
                        pa, pb = finalize(h, sc, st["ao"], st["dcolT"])
                        st["tp"] = pa()
                        st["pb"] = pb
                    def fin_b(st=st):
                        st["pb"](st["tp"])
                    yield fin_a
                    yield fin_b

            def weave(proj_ops, b_ops, start_offset):
                """Emit proj_ops in order, interleaving b_ops evenly
                starting after start_offset proj ops."""
                b_ops = list(b_ops)
                npr, nb = len(proj_ops), len(b_ops)
                if nb == 0:
                    for op in proj_ops:
                        op()
                    return
                span = max(1, npr - start_offset)
                # b op i goes after proj op start_offset + i*span/nb
                bi = 0
                for n, op in enumerate(proj_ops):
                    op()
                    while bi < nb and n >= start_offset + bi * span // nb:
                        b_ops[bi]()
                        bi += 1
                while bi < nb:
                    b_ops[bi]()
                    bi += 1

            # ---------------- chunk 0: k-step-outer projection ----------
            xt0 = None
            qTcs = {}

            def c0_granule(g):
                ksl = slice(4 * g, 4 * g + 4)
                nc.sync.dma_start(w6_sb3[:, ksl, :], w63[:, ksl, :])
                if g == 2:
                    nc.sync.dma_start(ropeA_sb[:], ropeA[:])
                    nc.sync.dma_start(ropeB_sb[:], ropeB[:])
                    nc.sync.dma_start(masks_sb[:], masks[:])
                    nc.sync.dma_start(ones_col_sb[:], ones_col[:])
                    nc.sync.dma_start(ones128_sb[:], ones128[:])
                    nc.sync.dma_start(ident_sb[:], ident[:])
                    nc.sync.dma_start(identf_sb[:], identf[:])
                    nc.sync.dma_start(sel4_sb[:], sel4[:])

            # interleave w6 + x0 granules (w6 slice first, x after)
            xt = {}
            xt0_3 = {}

            def c0_load():
                x = load_x(0, granules=None)
                return x

            # manual interleave: w6 granule g then x granule g
            xt0 = xt_p.tile([P, NKT * SC], BF16, tag="xt", name="xt0")
            xt0v = xt0[:].rearrange("p (t m) -> p t m", m=SC)
            # granule ramp cuts time-to-first-matmul; w6 rides SP, x rides
            # the ACT HWDGE queue so their dma_start sequencing overlaps.
            # First w6 piece is just the k|v columns of k-tile 0 (the first
            # matmul only needs the k slice).
            nc.sync.dma_start(w6_sb3[:, 0:1, 0:2 * DH], w63[:, 0:1, 0:2 * DH])
            nc.scalar.dma_start(xt0v[:, 0:1, :], xT3[:, 0:1, 0:SC])
            nc.sync.dma_start(w6_sb3[:, 0:1, 2 * DH:], w63[:, 0:1, 2 * DH:])
            gsl = [slice(1, 2), slice(2, 4)] + \
                  [slice(4 + 2 * g, 6 + 2 * g) for g in range(14)]
            for gi, ksl in enumerate(gsl):
                nc.sync.dma_start(w6_sb3[:, ksl, :], w63[:, ksl, :])
                nc.scalar.dma_start(xt0v[:, ksl, :], xT3[:, ksl, 0:SC])
            # tables after all granules: ropeB/A first (needed by the first
            # evac ~2us after they land); masks/consts not until B(0)
            nc.sync.dma_start(ropeB_sb[:], ropeB[:])
            nc.sync.dma_start(ropeA_sb[:], ropeA[:])
            nc.sync.dma_start(masks_sb[:], masks[:])
            nc.sync.dma_start(ones_col_sb[:], ones_col[:])
            nc.sync.dma_start(ones128_sb[:], ones128[:])
            nc.sync.dma_start(ident_sb[:], ident[:])
            nc.sync.dma_start(identf_sb[:], identf[:])
            nc.sync.dma_start(sel4_sb[:], sel4[:])

            qTc0 = qTc_p.tile([P, NH * SC], BF16, tag="qTc", name="qTc0")
            qTcs[0] = qTc0
            # psums: k,v,q0 -> pacc; q1,q2 -> pS; q3 -> pao
            ps0 = {
                "k": pacc.tile([P, SC], F32, tag="acc", name="c0k"),
                "v": pacc.tile([P, SC], F32, tag="acc", name="c0v"),
                "q0": pacc.tile([P, SC], F32, tag="acc", name="c0q0"),
                "q1": pS.tile([P, SC], F32, tag="S", name="c0q1"),
                "q2": pS.tile([P, SC], F32, tag="S", name="c0q2"),
                "q3": pao.tile([P, SC], F32, tag="ao", name="c0q3"),
            }

            def c0_mm(j, kt):
                w_ap = w6_sb[:, kt * W6 + WOFF[j]:kt * W6 + WOFF[j] + DH]
                nc.tensor.matmul(ps0[j][:], w_ap,
                                 xt0[:, kt * SC:(kt + 1) * SC],
                                 start=(kt == 0), stop=(kt == NKT - 1))

            TAIL = 8  # last k-tiles run output-major so evacs start early
            for kt in range(NKT - TAIL):
                for j in OUTS:
                    c0_mm(j, kt)
            # tail: finish outputs one by one, evac immediately
            for j in ("k", "q0", "v", "q1", "q2", "q3"):
                for kt in range(NKT - TAIL, NKT):
                    c0_mm(j, kt)
                evac(j, 0, ps0[j], qTc0)

            # ---------------- chunks 1..3 + woven B(sc-1) ---------------
            xt_next = load_x(1)
            xts = {1: xt_next}

            for sc in range(1, NSC):
                xtc = xts[sc]
                qTc = qTc_p.tile([P, NH * SC], BF16, tag="qTc",
                                 name=f"qTc{sc}")
                qTcs[sc] = qTc
                proj_ops = []

                def add_output(j, sc=sc, xtc=xtc, qTc=qTc):
                    ps = pacc.tile([P, SC], F32, tag="acc",
                                   name=f"c{sc}{j}")
                    for kt in range(NKT):
                        def mm(j=j, kt=kt, ps=ps, sc=sc, xtc=xtc,
                               qTc=qTc):
                            w_ap = w6_sb[:, kt * W6 + WOFF[j]:
                                         kt * W6 + WOFF[j] + DH]
                            nc.tensor.matmul(
                                ps[:], w_ap, xtc[:, kt * SC:(kt + 1) * SC],
                                start=(kt == 0), stop=(kt == NKT - 1))
                            if kt == NKT - 1:
                                evac(j, sc, ps, qTc)
                        proj_ops.append(mm)

                # prefetch next chunk's x / wo right at chunk start
                def prefetch(sc=sc):
                    if sc + 1 < NSC:
                        xts[sc + 1] = load_x(sc + 1)
                    else:
                        wo_t = xt_p.tile([P, 4 * DIM], BF16, tag="xt",
                                         name="wo_t")
                        nc.sync.dma_start(
                            wo_t[:].rearrange("p (t m) -> p t m", m=DIM),
                            wo.rearrange("(t p) m -> p t m", p=P))
                        xts["wo"] = wo_t
                first = proj_ops_head = []
                for j in OUTS:
                    add_output(j)
                prefetch()
                weave(proj_ops, B_stream(sc - 1, qTcs[sc - 1]),
                      start_offset=32 if sc == 1 else 12)

            # ---------------- B(3) woven with phase C -------------------
            inner_close_done = False
            wo_t = xts["wo"]

            c_ops = []

            def add_C_st(st):
                ot = rbs_p.tile([P, DIM], BF16, tag="ot", bufs=2,
                                name=f"ot{st}")
                last = st == SEQ // P - 1
                for dc in range(8):
                    def grp(st=st, dc=dc, ot=ot, last=last):
                        if last and dc == 7:
                            # final dc: two independent [128,256] psum
                            # groups so the chain after the very last
                            # matmul is short
                            for q in range(2):
                                poh = pacc.tile([P, 256], F32, tag="acc",
                                                name=f"po{st}_{dc}_{q}")
                                for h in range(NH):
                                    nc.tensor.matmul(
                                        poh[:],
                                        aoT_sb[:, h * SEQ + st * P:
                                               h * SEQ + (st + 1) * P],
                                        wo_t[:, h * DIM + dc * SC + q * 256:
                                             h * DIM + dc * SC + (q + 1) * 256],
                                        start=(h == 0), stop=(h == NH - 1))
                                qs = dc * SC + q * 256
                                nc.vector.tensor_copy(ot[:, qs:qs + 256],
                                                      poh[:])
                                nc.sync.dma_start(
                                    out[st * P:(st + 1) * P, qs:qs + 256],
                                    ot[:, qs:qs + 256])
                            return
                        po = pacc.tile([P, SC], F32, tag="acc",
                                       name=f"po{st}_{dc}")
                        for h in range(NH):
                            nc.tensor.matmul(
                                po[:],
                                aoT_sb[:, h * SEQ + st * P:
                                       h * SEQ + (st + 1) * P],
                                wo_t[:, h * DIM + dc * SC:
                                     h * DIM + (dc + 1) * SC],
                                start=(h == 0), stop=(h == NH - 1))
                        nc.vector.tensor_copy(
                            ot[:, dc * SC:(dc + 1) * SC], po[:])
                        if last and dc in (3, 6):
                            # two bulk drains; the final dc pieces follow
                            lo = 0 if dc == 3 else 4 * SC
                            nc.sync.dma_start(
                                out[st * P:(st + 1) * P, lo:(dc + 1) * SC],
                                ot[:, lo:(dc + 1) * SC])
                        elif dc == 7:
                            nc.sync.dma_start(
                                out[st * P:(st + 1) * P, :], ot[:])
                    c_ops.append(grp)

            # st 0..11 depend only on B(0..2); weave B(3) into them
            for st in range(12):
                add_C_st(st)
            weave(c_ops, B_stream(NSC - 1, qTcs[NSC - 1]), start_offset=0)
            c_ops = []
            for st in range(12, 16):
                add_C_st(st)
            for op in c_ops:
                op()
            inner.close()
    nc.compile()
    return nc


def make_in_maps(x, freqs_cos, freqs_sin, wq, wk, wv, wo):
    """Host-side sharding + layout prep. Returns list of 8 per-core dicts."""
    import ml_dtypes
    bf16 = np.dtype(ml_dtypes.bfloat16)
    f32 = np.float32
    x2 = np.asarray(x, f32).reshape(SEQ, DIM)
    xT = np.ascontiguousarray(x2.T).astype(bf16)
    # RoPE de-interleave permutation within each head: evens then odds
    perm = np.concatenate([np.arange(0, DH, 2), np.arange(1, DH, 2)])
    scale = 1.0 / np.sqrt(np.float32(DH))
    cosT = np.ascontiguousarray(np.asarray(freqs_cos, f32).T)   # [64, SEQ]
    sinT = np.ascontiguousarray(np.asarray(freqs_sin, f32).T)
    ropeA = np.concatenate([cosT, cosT], axis=0)                # [128, SEQ]
    ropeB = np.concatenate([-sinT, sinT], axis=0)
    kk = np.arange(P)[:, None]
    qq = np.arange(P)[None, :]
    masks = np.where(qq - kk >= 0, 0.0, NEG).astype(bf16)
    ones_col = np.ones((1, P), f32)
    ones128 = np.ones((P, 1), bf16)
    ident = np.eye(P, dtype=bf16)

    wq_f = np.asarray(wq, f32)
    wk_f = np.asarray(wk, f32)
    wv_f = np.asarray(wv, f32)
    wo_f = np.asarray(wo, f32)
    in_maps = []
    for c in range(NCORES):
        wq_c = wq_f[:, c * DQ:(c + 1) * DQ].reshape(DIM, NH, DH)[:, :, perm]
        wq_c = wq_c.reshape(DIM, DQ) * scale
        wk_c = wk_f[:, c * DH:(c + 1) * DH][:, perm]
        wv_c = wv_f[:, c * DH:(c + 1) * DH]
        w6_c = np.ascontiguousarray(
            np.concatenate([wk_c, wv_c, wq_c], axis=1)).astype(bf16)
        wo_c = np.ascontiguousarray(wo_f[c * DQ:(c + 1) * DQ, :]).astype(bf16)
        in_maps.append({
            "xT": xT, "w6": w6_c, "wo": wo_c,
            "ropeA": ropeA, "ropeB": ropeB, "masks": masks,
            "ones_col": ones_col, "ones128": ones128, "ident": ident,
            "identf": np.eye(P, dtype=np.float16),
            "sel4": np.kron(np.eye(4, dtype=np.float16),
                            np.ones((1, P), np.float16)),
        })
    return in_maps


_NC_CACHE = None


def kernel(x, freqs_cos, freqs_sin, mask, wq, wk, wv, wo):
    """Full-input entry point: returns [1, 2048, 4096] float32."""
    global _NC_CACHE
    from concourse.bass_utils import run_bass_kernel_spmd
    if _NC_CACHE is None:
        _NC_CACHE = build_nc()
    in_maps = make_in_maps(x, freqs_cos, freqs_sin, wq, wk, wv, wo)
    res = run_bass_kernel_spmd(_NC_CACHE, in_maps, core_ids=list(range(NCORES)))
    acc = np.zeros((SEQ, DIM), np.float32)
    for c in range(NCORES):
        acc += res.results[c]["out"].astype(np.float32)
    return acc.reshape(BS, SEQ, DIM)
